# revision 19
# baseline (speedup 1.0000x reference)
"""GatedGCN message-passing kernel for 8 TRN2 NeuronCores (Bass/Tile).

Sharding: core c owns the contiguous dst-node range [c*npc, (c+1)*npc) and all
edges whose dst falls in it.  Segment sums run per 128-node block as one-hot
matmuls on the TensorEngine (edges host-sorted by dst block); the src-side
gather uses dma_gather from a per-layer [N, 128] bf16 table of [Eh|Bh] that is
AllGathered across cores every layer.  dma_gather indices are int16, so edges
in each block are split into src<HALF / src>=HALF groups that gather from the
two halves of the table.
"""

import numpy as np

HID = 64
MLP = 128
N_CORES = 8
HALF_CAP = 32768
GCHUNK = 24
GST_BUFS = 6
KSET = 4
DBG_NO_COLLECTIVE = False
DBG_NO_GATHER = False


def _cfg(n_nodes, n_edges, n_layers):
    npc = n_nodes // N_CORES
    assert npc * N_CORES == n_nodes
    nblk = (npc + 127) // 128
    half = min(HALF_CAP, n_nodes)
    return dict(n_nodes=n_nodes, n_edges=n_edges, L=n_layers, npc=npc,
                nblk=nblk, half=half)


# ---------------------------------------------------------------------------
# host-side prep
# ---------------------------------------------------------------------------

def _prep(cfg, src, dst, e_feat):
    f16 = np.float16
    npc, nblk, half = cfg["npc"], cfg["nblk"], cfg["half"]
    n_cores = N_CORES

    src = np.asarray(src).astype(np.int64)
    dst = np.asarray(dst).astype(np.int64)
    e_feat = np.asarray(e_feat, np.float32)

    core_of = np.minimum(dst // npc, n_cores - 1)
    counts = np.zeros((n_cores, nblk, 2), np.int64)
    edge_ids = [[[None, None] for _ in range(nblk)] for _ in range(n_cores)]
    for c in range(n_cores):
        in_c = np.where(core_of == c)[0]
        dloc = dst[in_c] - c * npc
        blk = dloc // 128
        grp = (src[in_c] >= half).astype(np.int64)
        order = np.lexsort((src[in_c], grp, blk))
        in_c = in_c[order]
        key = blk[order] * 2 + grp[order]
        bounds = np.searchsorted(key, np.arange(nblk * 2 + 1))
        for b in range(nblk):
            for g in range(2):
                lo, hi = bounds[b * 2 + g], bounds[b * 2 + g + 1]
                edge_ids[c][b][g] = in_c[lo:hi]
                counts[c, b, g] = hi - lo

    T = (counts.max(axis=0) + 127) // 128          # [nblk, 2] tiles per group
    ntiles = int(T.sum())
    assert ntiles > 0

    # Slot/tile order: sets of KSET blocks, group-major inside each set, so
    # gather calls merge across blocks (one dma_gather per same-group run).
    tile_blk = np.empty(ntiles, np.int32)
    tile_grp = np.empty(ntiles, np.int32)
    tile0_bg = np.empty((nblk, 2), np.int32)       # first tile of each (b,g)
    t = 0
    for s0 in range(0, nblk, KSET):
        for g in range(2):
            for b in range(s0, min(s0 + KSET, nblk)):
                tile0_bg[b, g] = t
                for _ in range(int(T[b, g])):
                    tile_blk[t] = b
                    tile_grp[t] = g
                    t += 1

    chunks = []                                     # (tile0, ntile, group)
    s = 0
    while s < ntiles:
        g = tile_grp[s]
        e = s
        while e < ntiles and tile_grp[e] == g and e - s < GCHUNK:
            e += 1
        chunks.append((int(s), int(e - s), int(g)))
        s = e

    # Processing groups in block-major order (all of b's tiles consecutively)
    # so each block's PSUM accumulation opens and closes without another
    # block's accumulation interleaving in the same bank.  Groups never
    # straddle a (b,g) run nor a gather-chunk boundary (one gst buffer per
    # group).
    chunk_end = np.empty(ntiles, np.int32)
    for (c0, cn, _g) in chunks:
        chunk_end[c0:c0 + cn] = c0 + cn
    groups = []   # (t0, gn, m23_off_cols)
    off = 0
    for b in range(nblk):
        for g in range(2):
            t = int(tile0_bg[b, g])
            end = t + int(T[b, g])
            while t < end:
                gn = min(4, end - t, int(chunk_end[t]) - t)
                groups.append((int(t), int(gn), int(off)))
                off += 2 * gn * 128
                t += gn
    plan = dict(T=T, ntiles=ntiles, tile_blk=tile_blk, tile_grp=tile_grp,
                chunks=chunks, groups=groups, m23_cols=int(off))

    E_pad = ntiles * 128
    rows = np.arange(E_pad) % 128
    tt = np.arange(E_pad) // 128
    per_core = []
    for c in range(n_cores):
        eid = np.zeros(E_pad, np.int64)
        valid = np.zeros(E_pad, bool)
        pos = 0
        for s0 in range(0, nblk, KSET):
            for g in range(2):
                for b in range(s0, min(s0 + KSET, nblk)):
                    ids = edge_ids[c][b][g]
                    n = len(ids)
                    eid[pos:pos + n] = ids
                    valid[pos:pos + n] = True
                    pos += int(T[b, g]) * 128
        esrc = src[eid].copy()
        edst = (dst[eid] - c * npc).copy()
        esrc[~valid] = 0
        edst[~valid] = 0

        bf16 = f16
        gi = esrc.copy()
        gi[gi >= half] -= half
        gi = gi.astype(np.int16).reshape(-1, 16).T
        gidx = np.ascontiguousarray(np.tile(gi, (8, 1)))        # [128, E_pad/16]

        doff = edst - tile_blk[tt].astype(np.int64) * 128
        ok = valid & (doff >= 0) & (doff < 128)
        m3 = np.zeros((ntiles, 128, 128), bf16)                  # [e, n]
        m3[tt[ok], rows[ok], doff[ok]] = 1.0
        m2f = m3.transpose(2, 0, 1).reshape(128, E_pad)          # [n, tiles*e]
        m3f = m3.transpose(1, 0, 2).reshape(128, E_pad)          # [e, tiles*n]
        # group-interleaved per the plan's group list: [m2 cols | m3 cols]
        import ml_dtypes
        m23 = np.zeros((128, plan["m23_cols"]), ml_dtypes.float8_e4m3)
        for (gt0, gg, goff) in plan["groups"]:
            a, b_ = gt0 * 128, (gt0 + gg) * 128
            m23[:, goff:goff + (b_ - a)] = m2f[:, a:b_]
            m23[:, goff + gg * 128:goff + 2 * gg * 128] = m3f[:, a:b_]

        ef = np.zeros((3, E_pad), f16)
        efv = e_feat[eid]
        efv[~valid] = 0.0
        ef[0, :] = efv[:, 0].astype(f16)
        ef[1, :] = efv[:, 1].astype(f16)
        ef[2, :] = valid.astype(f16)

        per_core.append(dict(gidx=gidx, m23=m23, e_feat_t=ef))
    return plan, per_core


def _weights(cfg, inputs):
    f16 = np.float16
    f32 = np.float32
    Lw = np.asarray(inputs["W_layers"], f32)
    Lb = np.asarray(inputs["b_layers"], f32)
    w_emb_e = np.asarray(inputs["W_emb_e"], f32)
    b_emb_e = np.asarray(inputs["b_emb_e"], f32)
    w = {}
    w["w_emb_h"] = np.concatenate(
        [np.asarray(inputs["W_emb_h"], f32),
         np.asarray(inputs["b_emb_h"], f32)[None, :]], 0)           # [7,64] f32
    w["w_emb_e"] = np.concatenate(
        [w_emb_e, b_emb_e[None, :]], 0).astype(f16)
    for l in range(cfg["L"]):
        A, B, D, E, C = (Lw[l, i] for i in range(5))
        bA, bB, bD, bE, bC = (Lb[l, i] for i in range(5))
        eb = np.zeros((HID + 1, 2 * HID), f32)
        eb[:HID, :HID] = E
        eb[:HID, HID:] = B
        eb[HID, HID:] = bB
        w[f"w_eb{l}"] = eb
        da = np.zeros((HID + 1, 2 * HID), f32)
        da[:HID, :HID] = D
        da[:HID, HID:] = A
        da[HID, :HID] = bD + bC + bE
        da[HID, HID:] = bA
        w[f"w_da{l}"] = da
        if l == 0:
            w["w4c0"] = np.concatenate(
                [w_emb_e @ C, (b_emb_e @ C)[None, :]], 0).astype(f16)
        else:
            w[f"w4_{l}"] = C.astype(f16)                           # [64,64]
    w["w1"] = np.concatenate(
        [np.asarray(inputs["W1"], f32),
         np.asarray(inputs["b1"], f32)[None, :]], 0)                # [65,128]
    w["w2"] = np.asarray(inputs["W2"], f32).astype(f16)            # [128,2]
    w["b2"] = np.asarray(inputs["b2"], f32).reshape(2, 1)           # [2,1]
    ident = np.eye(128)
    w["id16"] = ident.astype(f16)
    w["id32"] = ident.astype(f32)
    return w


# ---------------------------------------------------------------------------
# device program
# ---------------------------------------------------------------------------

def _build(cfg, plan):
    import concourse.bacc as bacc
    import concourse.mybir as mybir
    from concourse import tile
    from contextlib import ExitStack

    f32 = mybir.dt.float32
    f16 = mybir.dt.float16
    i16 = mybir.dt.int16
    AF = mybir.ActivationFunctionType
    ALU = mybir.AluOpType

    L = cfg["L"]
    npc, nblk, half = cfg["npc"], cfg["nblk"], cfg["half"]
    ntiles = plan["ntiles"]
    tile_blk = plan["tile_blk"]
    chunks = plan["chunks"]
    T = plan["T"]
    E_pad = ntiles * 128
    NP = nblk * 128
    n_tab = npc * N_CORES

    nc = bacc.Bacc("TRN2", target_bir_lowering=False, debug=False,
                   num_devices=N_CORES, num_swdge_queues=4)

    wshapes = {
        "w_emb_h": ([7, HID], f32), "w_emb_e": ([3, HID], f16),
        "w4c0": ([3, HID], f16), "w1": ([HID + 1, MLP], f32),
        "w2": ([MLP, 2], f16), "b2": ([2, 1], f32),
        "id16": ([128, 128], f16), "id32": ([128, 128], f32),
    }
    for l in range(L):
        wshapes[f"w_eb{l}"] = ([HID + 1, 2 * HID], f32)
        wshapes[f"w_da{l}"] = ([HID + 1, 2 * HID], f32)
        if l > 0:
            wshapes[f"w4_{l}"] = ([HID, HID], f16)

    p_hfeat = nc.declare_dram_parameter("h_feat_t", [7, NP], f32, isOutput=False)
    p_efeat = nc.declare_dram_parameter("e_feat_t", [3, E_pad], f16, isOutput=False)
    p_gidx = nc.declare_dram_parameter("gidx", [128, E_pad // 16], i16, isOutput=False)
    p_m23 = nc.declare_dram_parameter("m23", [128, plan["m23_cols"]], mybir.dt.float8e4, isOutput=False)
    p_w = {k: nc.declare_dram_parameter(k, s, d, isOutput=False)
           for k, (s, d) in wshapes.items()}
    p_out = nc.declare_dram_parameter("out", [2, NP], f32, isOutput=True)

    eb_own = [nc.dram_tensor(f"eb_own{i}", [npc, 2 * HID], f16) for i in range(2)]
    eb_tab = [nc.dram_tensor(f"eb_tab{i}", [n_tab, 2 * HID], f16,
                             addr_space="Shared") for i in range(2)]
    e_buf = [nc.dram_tensor(f"e_buf{i}", [HID, E_pad], f16)
             for i in range(2)]
    rg = [list(range(N_CORES))]

    with tile.TileContext(nc) as tc, ExitStack() as ctx:
        const = ctx.enter_context(tc.tile_pool(name="const", bufs=1))
        persist = ctx.enter_context(tc.tile_pool(name="persist", bufs=1))
        sw = ctx.enter_context(tc.tile_pool(name="sw", bufs=4))
        gst = ctx.enter_context(tc.tile_pool(name="gst", bufs=GST_BUFS))
        blkp = ctx.enter_context(tc.tile_pool(name="blkp", bufs=2))
        ps_eh = ctx.enter_context(tc.tile_pool(name="ps_eh", bufs=2, space="PSUM"))
        ps_sc = ctx.enter_context(tc.tile_pool(name="ps_sc", bufs=2, space="PSUM"))
        ps_tr = ctx.enter_context(tc.tile_pool(name="ps_tr", bufs=2, space="PSUM"))
        ps_bk = ctx.enter_context(tc.tile_pool(name="ps_bk", bufs=2, space="PSUM"))

        wsb = {}
        for k, (s, d) in wshapes.items():
            t_ = const.tile(s, d, tag=f"w_{k}")
            nc.sync.dma_start(out=t_[:], in_=p_w[k][:, :])
            wsb[k] = t_

        gidx_sb = persist.tile([128, E_pad // 16], i16)
        nc.sync.dma_start(out=gidx_sb[:, :], in_=p_gidx[:, :])

        h_sb = persist.tile([128, nblk * HID], f32)
        ht_sb = persist.tile([HID + 1, NP], f32)
        nc.vector.memset(ht_sb[HID:HID + 1, :], 1.0)
        hfeat_sb = persist.tile([7, NP], f32)
        nc.sync.dma_start(out=hfeat_sb[:, :], in_=p_hfeat[:, :])

        def ht_block(b):
            return ht_sb[:, b * 128:(b + 1) * 128]

        def transpose_h_and_table(b, l):
            trp = ps_tr.tile([HID, 512], f32, tag="tr")
            nc.tensor.transpose(trp[:, 0:128], h_sb[:, b * HID:(b + 1) * HID],
                                wsb["id32"][:, :])
            nc.scalar.activation(ht_sb[0:HID, b * 128:(b + 1) * 128],
                                 trp[:, 0:128], AF.Copy)
            if l < L:
                ebp = ps_bk.tile([128, 128], f32, tag="bk")
                nc.tensor.matmul(ebp[:, :], ht_block(b), wsb[f"w_eb{l}"][:],
                                 start=True, stop=True, skip_group_check=True)
                ebs = blkp.tile([128, 2 * HID], f16, tag="ebs")
                nc.scalar.activation(ebs[:, :], ebp[:, :], AF.Copy)
                ne = min(128, npc - b * 128)
                nc.sync.dma_start(out=eb_own[l % 2][b * 128:b * 128 + ne, :],
                                  in_=ebs[0:ne, :])

        def head_block(b):
            z1p = ps_bk.tile([128, 128], f32, tag="bk")
            nc.tensor.matmul(z1p[:, :], wsb["w1"][:], ht_block(b),
                             start=True, stop=True, skip_group_check=True)
            z1 = blkp.tile([MLP, 128], f16, tag="z1s")
            nc.scalar.activation(z1[:, :], z1p[:, :], AF.Relu)
            z2p = ps_tr.tile([HID, 512], f32, tag="tr")
            nc.tensor.matmul(z2p[0:2, 0:128], wsb["w2"][:], z1[:, :],
                             start=True, stop=True, skip_group_check=True)
            th = blkp.tile([2, 128], f32, tag="th")
            nc.scalar.activation(th[:, :], z2p[0:2, 0:128], AF.Tanh,
                                 bias=wsb["b2"][:, 0:1])
            out_sb = blkp.tile([2, 128], f32, tag="outs")
            nc.vector.tensor_scalar_mul(out_sb[:, :], th[:, :], -1.2)
            nc.sync.dma_start(out=p_out[:, b * 128:(b + 1) * 128], in_=out_sb[:, :])

        def update_block(b, l, sc, ah):
            hb = h_sb[:, b * HID:(b + 1) * HID]
            den = blkp.tile([128, HID], f32, tag="den")
            nc.scalar.activation(den[:, :], sc[:, HID:], AF.Copy, bias=1e-6)
            rec = blkp.tile([128, HID], f32, tag="rec")
            nc.vector.reciprocal(rec[:, :], den[:, :])
            div = blkp.tile([128, HID], f32, tag="div")
            nc.vector.tensor_mul(div[:, :], sc[:, 0:HID], rec[:, :])
            pre = blkp.tile([128, HID], f32, tag="pre")
            nc.vector.tensor_add(pre[:, :], div[:, :], ah[:, :])
            rl = blkp.tile([128, HID], f32, tag="rl")
            nc.scalar.activation(rl[:, :], pre[:, :], AF.Relu)
            nc.vector.tensor_add(hb, hb, rl[:, :])
            transpose_h_and_table(b, l + 1)
            if l + 1 == L:
                head_block(b)

        # ---- layer 0: h embedding + transposed copy + EB table -----------
        for b in range(nblk):
            ps = ps_bk.tile([128, 128], f32, tag="bk")
            nc.tensor.matmul(ps[:, 0:HID], hfeat_sb[:, b * 128:(b + 1) * 128],
                             wsb["w_emb_h"][:], start=True, stop=True,
                             skip_group_check=True)
            nc.scalar.activation(h_sb[:, b * HID:(b + 1) * HID], ps[:, 0:HID],
                                 AF.Copy)
            transpose_h_and_table(b, 0)

        def allgather(l=0):
            if DBG_NO_COLLECTIVE:
                cp = blkp.tile([128, 2 * HID], f16, tag="agcp", name=f"agcp{len(ag_n)}")
                ag_n.append(1)
                nc.sync.dma_start(out=cp[:, :], in_=eb_own[l % 2][0:128, :])
                nc.sync.dma_start(out=eb_tab[l % 2][0:128, :], in_=cp[:, :])
                return
            nc.gpsimd.collective_compute(
                "AllGather", ALU.bypass, replica_groups=rg,
                ins=[eb_own[l % 2][:, :].opt()], outs=[eb_tab[l % 2][:, :].opt()])
        ag_n = []

        allgather(0)

        # ---- layer sweeps -------------------------------------------------
        for l in range(L):
            g_of_tile = {}
            for ci, (t0, tn, grp) in enumerate(chunks):
                g = gst.tile([128, GCHUNK, 128], f16, tag="gather")
                base = half if grp else 0
                nrows = (n_tab - half) if grp else half
                if DBG_NO_GATHER:
                    nc.vector.memset(g[:, 0:tn, :], 0.125)
                else:
                    nc.gpsimd.dma_gather(
                        out_ap=g[:, 0:tn, :],
                        in_ap=eb_tab[l % 2][base:base + nrows, :],
                        idxs_ap=gidx_sb[:, t0 * 8:(t0 + tn) * 8],
                        num_idxs=tn * 128,
                        num_idxs_reg=tn * 128,
                        elem_size=2 * HID,
                        single_packet=False,
                        queue_num=ci % 4,
                    )
                for j in range(tn):
                    g_of_tile[t0 + j] = (g, j)

            dh_of_blk, ah_of_blk, sc_of_blk = {}, {}, {}
            sc_set_of = {}
            for (t, gn, goff) in plan["groups"]:
                c0, c1 = t * 128, (t + gn) * 128
                gtile, gj0 = g_of_tile[t]
                ehp = ps_eh.tile([128, 4, HID], f32, tag="ehat")
                et_sb = sw.tile([HID, 512], f16, tag="et")
                if l == 0:
                    ef_sb = sw.tile([3, 512], f16, tag="ef")
                    nc.sync.dma_start(out=ef_sb[:, 0:gn * 128],
                                      in_=p_efeat[:, c0:c1])
                    eemb_ps = ps_tr.tile([HID, 512], f32, tag="tr")
                else:
                    nc.sync.dma_start(out=et_sb[:, 0:gn * 128],
                                      in_=e_buf[(l - 1) % 2][:, c0:c1])
                m23_sb = sw.tile([128, 1024], mybir.dt.float8e4, tag="m23")
                nc.sync.dma_start(out=m23_sb[:, 0:2 * gn * 128],
                                  in_=p_m23[:, goff:goff + 2 * gn * 128])
                v_sb = sw.tile([128, 4, 128], f16, tag="v")
                if l < L - 1:
                    etr_ps = ps_tr.tile([HID, 512], f16, tag="tr")
                    relu_sb = sw.tile([128, 4, HID], f16, tag="relu")
                    enx_sb = sw.tile([HID, 512], f16, tag="enx")

                for j in range(gn):
                    tj = t + j
                    b = int(tile_blk[tj])
                    if b not in dh_of_blk:
                        dap = ps_bk.tile([128, 128], f32, tag="bk")
                        nc.tensor.matmul(dap[:, :], ht_block(b),
                                         wsb[f"w_da{l}"][:], start=True,
                                         stop=True, skip_group_check=True)
                        dh = blkp.tile([128, HID], f16, tag="dh", bufs=3)
                        nc.scalar.activation(dh[:, :], dap[:, 0:HID], AF.Copy)
                        ah = blkp.tile([128, HID], f32, tag="ah", bufs=3)
                        nc.scalar.activation(ah[:, :], dap[:, HID:], AF.Copy)
                        dh_of_blk[b] = dh
                        ah_of_blk[b] = ah
                        sc_of_blk[b] = [ps_sc.tile([128, 128], f32, tag="sc",
                                                   name=f"sc_{l}_{b}"), 0]

                    if l == 0:
                        nc.tensor.matmul(ehp[:, j, :], ef_sb[:, j * 128:(j + 1) * 128],
                                         wsb["w4c0"][:], start=True, stop=False,
                                         skip_group_check=True)
                        nc.tensor.matmul(eemb_ps[:, j * 128:(j + 1) * 128],
                                         wsb["w_emb_e"][:],
                                         ef_sb[:, j * 128:(j + 1) * 128],
                                         start=True, stop=True,
                                         skip_group_check=True)
                    else:
                        nc.tensor.matmul(ehp[:, j, :], et_sb[:, j * 128:(j + 1) * 128],
                                         wsb[f"w4_{l}"][:], start=True,
                                         stop=False, skip_group_check=True)
                    nc.tensor.matmul(ehp[:, j, :], m23_sb[:, j * 128:(j + 1) * 128],
                                     dh_of_blk[b][:, :], start=False, stop=False,
                                     skip_group_check=True)
                    nc.tensor.matmul(ehp[:, j, :], wsb["id16"][:],
                                     gtile[:, gj0 + j, 0:HID], start=False, stop=True,
                                     skip_group_check=True)
                if l == 0:
                    nc.scalar.activation(et_sb[:, 0:gn * 128],
                                         eemb_ps[:, 0:gn * 128], AF.Copy)
                # batched sigma / mul over the group
                nc.scalar.activation(v_sb[:, 0:gn, HID:], ehp[:, 0:gn, :],
                                     AF.Sigmoid)
                nc.vector.tensor_mul(v_sb[:, 0:gn, 0:HID], v_sb[:, 0:gn, HID:],
                                     gtile[:, gj0:gj0 + gn, HID:])
                if l < L - 1:
                    nc.scalar.activation(relu_sb[:, 0:gn, :], ehp[:, 0:gn, :],
                                         AF.Relu)
                for j in range(gn):
                    tj = t + j
                    b = int(tile_blk[tj])
                    sc, nmm = sc_of_blk[b]
                    total = int(T[b, 0]) + int(T[b, 1])
                    nc.tensor.matmul(sc[:, :],
                                     m23_sb[:, gn * 128 + j * 128:gn * 128 + (j + 1) * 128],
                                     v_sb[:, j, :],
                                     start=(nmm == 0), stop=(nmm == total - 1),
                                     skip_group_check=True)
                    sc_of_blk[b][1] = nmm + 1
                    if l < L - 1:
                        nc.tensor.transpose(etr_ps[:, j * 128:(j + 1) * 128],
                                            relu_sb[:, j, :], wsb["id16"][:, :])
                    if sc_of_blk[b][1] == total:
                        update_block(b, l, sc, ah_of_blk[b])
                        del dh_of_blk[b], ah_of_blk[b], sc_of_blk[b]

                if l < L - 1:
                    nc.vector.tensor_add(enx_sb[:, 0:gn * 128],
                                         et_sb[:, 0:gn * 128],
                                         etr_ps[:, 0:gn * 128])
                    nc.sync.dma_start(out=e_buf[l % 2][:, c0:c1],
                                      in_=enx_sb[:, 0:gn * 128])

            if l < L - 1:
                allgather(l + 1)

    nc.compile()
    return nc


# ---------------------------------------------------------------------------
# entry point
# ---------------------------------------------------------------------------

_CACHE = {}


def kernel(**inputs):
    from concourse.bass_utils import run_bass_kernel_spmd

    h_feat = np.asarray(inputs["h_feat"], np.float32)
    e_feat = np.asarray(inputs["e_feat"], np.float32)
    src = np.asarray(inputs["src"])
    dst = np.asarray(inputs["dst"])
    n_nodes = h_feat.shape[0]
    n_edges = e_feat.shape[0]
    n_layers = int(np.asarray(inputs["W_layers"]).shape[0])
    cfg = _cfg(n_nodes, n_edges, n_layers)

    plan, per_core = _prep(cfg, src, dst, e_feat)
    w = _weights(cfg, inputs)

    key = ("prog", n_nodes, n_edges, n_layers, plan["ntiles"],
           tuple(plan["tile_blk"].tolist()),
           tuple(plan["chunks"]), plan["m23_cols"])
    if key not in _CACHE:
        _CACHE[key] = _build(cfg, plan)
    nc = _CACHE[key]

    npc, nblk = cfg["npc"], cfg["nblk"]
    NP = nblk * 128
    in_maps = []
    for c in range(N_CORES):
        hft = np.zeros((7, NP), np.float32)
        sl = h_feat[c * npc:(c + 1) * npc]
        hft[0:6, 0:npc] = sl.T
        hft[6, 0:npc] = 1.0
        m = per_core[c]
        im = {"h_feat_t": hft, "e_feat_t": m["e_feat_t"], "gidx": m["gidx"],
              "m23": m["m23"]}
        im.update(w)
        in_maps.append(im)

    res = run_bass_kernel_spmd(nc, in_maps, core_ids=list(range(N_CORES)))
    out = np.empty((n_nodes, 2), np.float32)
    for c in range(N_CORES):
        out[c * npc:(c + 1) * npc] = res.results[c]["out"][:, 0:npc].T
    kernel.last_results = res
    return out



# revision 21
# speedup vs baseline: 1.0144x; 1.0144x over previous
"""GatedGCN message-passing kernel for 8 TRN2 NeuronCores (Bass/Tile).

Sharding: core c owns the contiguous dst-node range [c*npc, (c+1)*npc) and all
edges whose dst falls in it.  Segment sums run per 128-node block as one-hot
matmuls on the TensorEngine (edges host-sorted by dst block); the src-side
gather uses dma_gather from a per-layer [N, 128] bf16 table of [Eh|Bh] that is
AllGathered across cores every layer.  dma_gather indices are int16, so edges
in each block are split into src<HALF / src>=HALF groups that gather from the
two halves of the table.
"""

import numpy as np

HID = 64
MLP = 128
N_CORES = 8
HALF_CAP = 32768
GCHUNK = 16
GST_BUFS = 4
KSET = 4
DBG_NO_COLLECTIVE = False
DBG_NO_GATHER = False


def _cfg(n_nodes, n_edges, n_layers):
    npc = n_nodes // N_CORES
    assert npc * N_CORES == n_nodes
    nblk = (npc + 127) // 128
    half = min(HALF_CAP, n_nodes)
    return dict(n_nodes=n_nodes, n_edges=n_edges, L=n_layers, npc=npc,
                nblk=nblk, half=half)


# ---------------------------------------------------------------------------
# host-side prep
# ---------------------------------------------------------------------------

def _prep(cfg, src, dst, e_feat):
    f16 = np.float16
    npc, nblk, half = cfg["npc"], cfg["nblk"], cfg["half"]
    n_cores = N_CORES

    src = np.asarray(src).astype(np.int64)
    dst = np.asarray(dst).astype(np.int64)
    e_feat = np.asarray(e_feat, np.float32)

    core_of = np.minimum(dst // npc, n_cores - 1)
    counts = np.zeros((n_cores, nblk, 2), np.int64)
    edge_ids = [[[None, None] for _ in range(nblk)] for _ in range(n_cores)]
    for c in range(n_cores):
        in_c = np.where(core_of == c)[0]
        dloc = dst[in_c] - c * npc
        blk = dloc // 128
        grp = (src[in_c] >= half).astype(np.int64)
        order = np.lexsort((src[in_c], grp, blk))
        in_c = in_c[order]
        key = blk[order] * 2 + grp[order]
        bounds = np.searchsorted(key, np.arange(nblk * 2 + 1))
        for b in range(nblk):
            for g in range(2):
                lo, hi = bounds[b * 2 + g], bounds[b * 2 + g + 1]
                edge_ids[c][b][g] = in_c[lo:hi]
                counts[c, b, g] = hi - lo

    T = (counts.max(axis=0) + 127) // 128          # [nblk, 2] tiles per group
    ntiles = int(T.sum())
    assert ntiles > 0

    # Slot/tile order: sets of KSET blocks, group-major inside each set, so
    # gather calls merge across blocks (one dma_gather per same-group run).
    tile_blk = np.empty(ntiles, np.int32)
    tile_grp = np.empty(ntiles, np.int32)
    tile0_bg = np.empty((nblk, 2), np.int32)       # first tile of each (b,g)
    t = 0
    for s0 in range(0, nblk, KSET):
        for g in range(2):
            for b in range(s0, min(s0 + KSET, nblk)):
                tile0_bg[b, g] = t
                for _ in range(int(T[b, g])):
                    tile_blk[t] = b
                    tile_grp[t] = g
                    t += 1

    chunks = []                                     # (tile0, ntile, group)
    s = 0
    while s < ntiles:
        g = tile_grp[s]
        e = s
        while e < ntiles and tile_grp[e] == g and e - s < GCHUNK:
            e += 1
        chunks.append((int(s), int(e - s), int(g)))
        s = e

    # Processing groups in block-major order (all of b's tiles consecutively)
    # so each block's PSUM accumulation opens and closes without another
    # block's accumulation interleaving in the same bank.  Groups never
    # straddle a (b,g) run nor a gather-chunk boundary (one gst buffer per
    # group).
    chunk_end = np.empty(ntiles, np.int32)
    for (c0, cn, _g) in chunks:
        chunk_end[c0:c0 + cn] = c0 + cn
    groups = []   # (t0, gn, m23_off_cols)
    off = 0
    for b in range(nblk):
        for g in range(2):
            t = int(tile0_bg[b, g])
            end = t + int(T[b, g])
            while t < end:
                gn = min(4, end - t, int(chunk_end[t]) - t)
                groups.append((int(t), int(gn), int(off)))
                off += 2 * gn * 128
                t += gn
    plan = dict(T=T, ntiles=ntiles, tile_blk=tile_blk, tile_grp=tile_grp,
                chunks=chunks, groups=groups, m23_cols=int(off))

    E_pad = ntiles * 128
    rows = np.arange(E_pad) % 128
    tt = np.arange(E_pad) // 128
    per_core = []
    for c in range(n_cores):
        eid = np.zeros(E_pad, np.int64)
        valid = np.zeros(E_pad, bool)
        pos = 0
        for s0 in range(0, nblk, KSET):
            for g in range(2):
                for b in range(s0, min(s0 + KSET, nblk)):
                    ids = edge_ids[c][b][g]
                    n = len(ids)
                    eid[pos:pos + n] = ids
                    valid[pos:pos + n] = True
                    pos += int(T[b, g]) * 128
        esrc = src[eid].copy()
        edst = (dst[eid] - c * npc).copy()
        esrc[~valid] = 0
        edst[~valid] = 0

        bf16 = f16
        gi = esrc.copy()
        gi[gi >= half] -= half
        gi = gi.astype(np.int16).reshape(-1, 16).T
        gidx = np.ascontiguousarray(np.tile(gi, (8, 1)))        # [128, E_pad/16]

        doff = edst - tile_blk[tt].astype(np.int64) * 128
        ok = valid & (doff >= 0) & (doff < 128)
        m3 = np.zeros((ntiles, 128, 128), bf16)                  # [e, n]
        m3[tt[ok], rows[ok], doff[ok]] = 1.0
        m2f = m3.transpose(2, 0, 1).reshape(128, E_pad)          # [n, tiles*e]
        m3f = m3.transpose(1, 0, 2).reshape(128, E_pad)          # [e, tiles*n]
        # group-interleaved per the plan's group list: [m2 cols | m3 cols]
        import ml_dtypes
        m23 = np.zeros((128, plan["m23_cols"]), ml_dtypes.float8_e4m3)
        for (gt0, gg, goff) in plan["groups"]:
            a, b_ = gt0 * 128, (gt0 + gg) * 128
            m23[:, goff:goff + (b_ - a)] = m2f[:, a:b_]
            m23[:, goff + gg * 128:goff + 2 * gg * 128] = m3f[:, a:b_]

        ef = np.zeros((3, E_pad), f16)
        efv = e_feat[eid]
        efv[~valid] = 0.0
        ef[0, :] = efv[:, 0].astype(f16)
        ef[1, :] = efv[:, 1].astype(f16)
        ef[2, :] = valid.astype(f16)

        per_core.append(dict(gidx=gidx, m23=m23, e_feat_t=ef))
    return plan, per_core


def _weights(cfg, inputs):
    f16 = np.float16
    f32 = np.float32
    Lw = np.asarray(inputs["W_layers"], f32)
    Lb = np.asarray(inputs["b_layers"], f32)
    w_emb_e = np.asarray(inputs["W_emb_e"], f32)
    b_emb_e = np.asarray(inputs["b_emb_e"], f32)
    w = {}
    w["w_emb_h"] = np.concatenate(
        [np.asarray(inputs["W_emb_h"], f32),
         np.asarray(inputs["b_emb_h"], f32)[None, :]], 0)           # [7,64] f32
    w["w_emb_e"] = np.concatenate(
        [w_emb_e, b_emb_e[None, :]], 0).astype(f16)
    for l in range(cfg["L"]):
        A, B, D, E, C = (Lw[l, i] for i in range(5))
        bA, bB, bD, bE, bC = (Lb[l, i] for i in range(5))
        eb = np.zeros((HID + 1, 2 * HID), f32)
        eb[:HID, :HID] = E
        eb[:HID, HID:] = B
        eb[HID, HID:] = bB
        w[f"w_eb{l}"] = eb
        da = np.zeros((HID + 1, 2 * HID), f32)
        da[:HID, :HID] = D
        da[:HID, HID:] = A
        da[HID, :HID] = bD + bC + bE
        da[HID, HID:] = bA
        w[f"w_da{l}"] = da
        if l == 0:
            w["w4c0"] = np.concatenate(
                [w_emb_e @ C, (b_emb_e @ C)[None, :]], 0).astype(f16)
        else:
            w[f"w4_{l}"] = C.astype(f16)                           # [64,64]
    w["w1"] = np.concatenate(
        [np.asarray(inputs["W1"], f32),
         np.asarray(inputs["b1"], f32)[None, :]], 0)                # [65,128]
    w["w2"] = np.asarray(inputs["W2"], f32).astype(f16)            # [128,2]
    w["b2"] = np.asarray(inputs["b2"], f32).reshape(2, 1)           # [2,1]
    ident = np.eye(128)
    w["id16"] = ident.astype(f16)
    w["id32"] = ident.astype(f32)
    return w


# ---------------------------------------------------------------------------
# device program
# ---------------------------------------------------------------------------

def _build(cfg, plan):
    import concourse.bacc as bacc
    import concourse.mybir as mybir
    from concourse import tile
    from contextlib import ExitStack

    f32 = mybir.dt.float32
    f16 = mybir.dt.float16
    i16 = mybir.dt.int16
    AF = mybir.ActivationFunctionType
    ALU = mybir.AluOpType

    L = cfg["L"]
    npc, nblk, half = cfg["npc"], cfg["nblk"], cfg["half"]
    ntiles = plan["ntiles"]
    tile_blk = plan["tile_blk"]
    chunks = plan["chunks"]
    T = plan["T"]
    E_pad = ntiles * 128
    NP = nblk * 128
    n_tab = npc * N_CORES

    nc = bacc.Bacc("TRN2", target_bir_lowering=False, debug=False,
                   num_devices=N_CORES, num_swdge_queues=4)

    wshapes = {
        "w_emb_h": ([7, HID], f32), "w_emb_e": ([3, HID], f16),
        "w4c0": ([3, HID], f16), "w1": ([HID + 1, MLP], f32),
        "w2": ([MLP, 2], f16), "b2": ([2, 1], f32),
        "id16": ([128, 128], f16), "id32": ([128, 128], f32),
    }
    for l in range(L):
        wshapes[f"w_eb{l}"] = ([HID + 1, 2 * HID], f32)
        wshapes[f"w_da{l}"] = ([HID + 1, 2 * HID], f32)
        if l > 0:
            wshapes[f"w4_{l}"] = ([HID, HID], f16)

    p_hfeat = nc.declare_dram_parameter("h_feat_t", [7, NP], f32, isOutput=False)
    p_efeat = nc.declare_dram_parameter("e_feat_t", [3, E_pad], f16, isOutput=False)
    p_gidx = nc.declare_dram_parameter("gidx", [128, E_pad // 16], i16, isOutput=False)
    p_m23 = nc.declare_dram_parameter("m23", [128, plan["m23_cols"]], mybir.dt.float8e4, isOutput=False)
    p_w = {k: nc.declare_dram_parameter(k, s, d, isOutput=False)
           for k, (s, d) in wshapes.items()}
    p_out = nc.declare_dram_parameter("out", [2, NP], f32, isOutput=True)

    eb_own = [nc.dram_tensor(f"eb_own{i}", [npc, 2 * HID], f16) for i in range(2)]
    eb_tab = [nc.dram_tensor(f"eb_tab{i}", [n_tab, 2 * HID], f16,
                             addr_space="Shared") for i in range(2)]
    e_buf = [nc.dram_tensor(f"e_buf{i}", [HID, E_pad], f16)
             for i in range(2)]
    rg = [list(range(N_CORES))]

    with tile.TileContext(nc) as tc, ExitStack() as ctx:
        const = ctx.enter_context(tc.tile_pool(name="const", bufs=1))
        persist = ctx.enter_context(tc.tile_pool(name="persist", bufs=1))
        sw = ctx.enter_context(tc.tile_pool(name="sw", bufs=4))
        gst = ctx.enter_context(tc.tile_pool(name="gst", bufs=GST_BUFS))
        blkp = ctx.enter_context(tc.tile_pool(name="blkp", bufs=2))
        ps_eh = ctx.enter_context(tc.tile_pool(name="ps_eh", bufs=2, space="PSUM"))
        ps_sc = ctx.enter_context(tc.tile_pool(name="ps_sc", bufs=2, space="PSUM"))
        ps_tr = ctx.enter_context(tc.tile_pool(name="ps_tr", bufs=2, space="PSUM"))
        ps_bk = ctx.enter_context(tc.tile_pool(name="ps_bk", bufs=2, space="PSUM"))

        wsb = {}
        for k, (s, d) in wshapes.items():
            t_ = const.tile(s, d, tag=f"w_{k}")
            nc.sync.dma_start(out=t_[:], in_=p_w[k][:, :])
            wsb[k] = t_

        gidx_sb = persist.tile([128, E_pad // 16], i16)
        nc.sync.dma_start(out=gidx_sb[:, :], in_=p_gidx[:, :])

        h_sb = persist.tile([128, nblk * HID], f32)
        ht_sb = persist.tile([HID + 1, NP], f32)
        nc.vector.memset(ht_sb[HID:HID + 1, :], 1.0)
        hfeat_sb = persist.tile([7, NP], f32)
        nc.sync.dma_start(out=hfeat_sb[:, :], in_=p_hfeat[:, :])

        def ht_block(b):
            return ht_sb[:, b * 128:(b + 1) * 128]

        def transpose_h_and_table(b, l):
            trp = ps_tr.tile([HID, 512], f32, tag="tr")
            nc.tensor.transpose(trp[:, 0:128], h_sb[:, b * HID:(b + 1) * HID],
                                wsb["id32"][:, :])
            nc.scalar.activation(ht_sb[0:HID, b * 128:(b + 1) * 128],
                                 trp[:, 0:128], AF.Copy)
            if l < L:
                ebp = ps_bk.tile([128, 128], f32, tag="bk")
                nc.tensor.matmul(ebp[:, :], ht_block(b), wsb[f"w_eb{l}"][:],
                                 start=True, stop=True, skip_group_check=True)
                ebs = blkp.tile([128, 2 * HID], f16, tag="ebs")
                nc.scalar.activation(ebs[:, :], ebp[:, :], AF.Copy)
                ne = min(128, npc - b * 128)
                nc.sync.dma_start(out=eb_own[l % 2][b * 128:b * 128 + ne, :],
                                  in_=ebs[0:ne, :])

        def head_block(b):
            z1p = ps_bk.tile([128, 128], f32, tag="bk")
            nc.tensor.matmul(z1p[:, :], wsb["w1"][:], ht_block(b),
                             start=True, stop=True, skip_group_check=True)
            z1 = blkp.tile([MLP, 128], f16, tag="z1s")
            nc.scalar.activation(z1[:, :], z1p[:, :], AF.Relu)
            z2p = ps_tr.tile([HID, 512], f32, tag="tr")
            nc.tensor.matmul(z2p[0:2, 0:128], wsb["w2"][:], z1[:, :],
                             start=True, stop=True, skip_group_check=True)
            th = blkp.tile([2, 128], f32, tag="th")
            nc.scalar.activation(th[:, :], z2p[0:2, 0:128], AF.Tanh,
                                 bias=wsb["b2"][:, 0:1])
            out_sb = blkp.tile([2, 128], f32, tag="outs")
            nc.vector.tensor_scalar_mul(out_sb[:, :], th[:, :], -1.2)
            nc.sync.dma_start(out=p_out[:, b * 128:(b + 1) * 128], in_=out_sb[:, :])

        def update_block(b, l, sc, ah):
            hb = h_sb[:, b * HID:(b + 1) * HID]
            den = blkp.tile([128, HID], f32, tag="den")
            nc.scalar.activation(den[:, :], sc[:, HID:], AF.Copy, bias=1e-6)
            rec = blkp.tile([128, HID], f32, tag="rec")
            nc.vector.reciprocal(rec[:, :], den[:, :])
            div = blkp.tile([128, HID], f32, tag="div")
            nc.vector.tensor_mul(div[:, :], sc[:, 0:HID], rec[:, :])
            pre = blkp.tile([128, HID], f32, tag="pre")
            nc.vector.tensor_add(pre[:, :], div[:, :], ah[:, :])
            rl = blkp.tile([128, HID], f32, tag="rl")
            nc.scalar.activation(rl[:, :], pre[:, :], AF.Relu)
            nc.vector.tensor_add(hb, hb, rl[:, :])
            transpose_h_and_table(b, l + 1)
            if l + 1 == L:
                head_block(b)

        # ---- layer 0: h embedding + transposed copy + EB table -----------
        for b in range(nblk):
            ps = ps_bk.tile([128, 128], f32, tag="bk")
            nc.tensor.matmul(ps[:, 0:HID], hfeat_sb[:, b * 128:(b + 1) * 128],
                             wsb["w_emb_h"][:], start=True, stop=True,
                             skip_group_check=True)
            nc.scalar.activation(h_sb[:, b * HID:(b + 1) * HID], ps[:, 0:HID],
                                 AF.Copy)
            transpose_h_and_table(b, 0)

        def allgather(l=0):
            if DBG_NO_COLLECTIVE:
                cp = blkp.tile([128, 2 * HID], f16, tag="agcp", name=f"agcp{len(ag_n)}")
                ag_n.append(1)
                nc.sync.dma_start(out=cp[:, :], in_=eb_own[l % 2][0:128, :])
                nc.sync.dma_start(out=eb_tab[l % 2][0:128, :], in_=cp[:, :])
                return
            nc.gpsimd.collective_compute(
                "AllGather", ALU.bypass, replica_groups=rg,
                ins=[eb_own[l % 2][:, :].opt()], outs=[eb_tab[l % 2][:, :].opt()])
        ag_n = []

        allgather(0)

        # ---- layer sweeps -------------------------------------------------
        for l in range(L):
            g_of_tile = {}
            for ci, (t0, tn, grp) in enumerate(chunks):
                g = gst.tile([128, GCHUNK, 128], f16, tag="gather")
                base = half if grp else 0
                nrows = (n_tab - half) if grp else half
                if DBG_NO_GATHER:
                    nc.vector.memset(g[:, 0:tn, :], 0.125)
                else:
                    nc.gpsimd.dma_gather(
                        out_ap=g[:, 0:tn, :],
                        in_ap=eb_tab[l % 2][base:base + nrows, :],
                        idxs_ap=gidx_sb[:, t0 * 8:(t0 + tn) * 8],
                        num_idxs=tn * 128,
                        num_idxs_reg=tn * 128,
                        elem_size=2 * HID,
                        single_packet=False,
                        queue_num=ci % 4,
                    )
                for j in range(tn):
                    g_of_tile[t0 + j] = (g, j)

            dh_of_blk, ah_of_blk, sc_of_blk = {}, {}, {}
            sc_set_of = {}
            for (t, gn, goff) in plan["groups"]:
                c0, c1 = t * 128, (t + gn) * 128
                gtile, gj0 = g_of_tile[t]
                ehp = ps_eh.tile([128, 4, HID], f32, tag="ehat")
                et_sb = sw.tile([HID, 512], f16, tag="et")
                if l == 0:
                    ef_sb = sw.tile([3, 512], f16, tag="ef")
                    nc.sync.dma_start(out=ef_sb[:, 0:gn * 128],
                                      in_=p_efeat[:, c0:c1])
                    eemb_ps = ps_tr.tile([HID, 512], f32, tag="tr")
                else:
                    nc.sync.dma_start(out=et_sb[:, 0:gn * 128],
                                      in_=e_buf[(l - 1) % 2][:, c0:c1])
                m23_sb = sw.tile([128, 1024], mybir.dt.float8e4, tag="m23")
                nc.sync.dma_start(out=m23_sb[:, 0:2 * gn * 128],
                                  in_=p_m23[:, goff:goff + 2 * gn * 128])
                v_sb = sw.tile([128, 4, 128], f16, tag="v")
                if l < L - 1:
                    etr_ps = ps_tr.tile([HID, 512], f16, tag="tr")
                    relu_sb = sw.tile([128, 4, HID], f16, tag="relu")
                    enx_sb = sw.tile([HID, 512], f16, tag="enx")

                for j in range(gn):
                    tj = t + j
                    b = int(tile_blk[tj])
                    if b not in dh_of_blk:
                        dap = ps_bk.tile([128, 128], f32, tag="bk")
                        nc.tensor.matmul(dap[:, :], ht_block(b),
                                         wsb[f"w_da{l}"][:], start=True,
                                         stop=True, skip_group_check=True)
                        dh = blkp.tile([128, HID], f16, tag="dh", bufs=3)
                        nc.scalar.activation(dh[:, :], dap[:, 0:HID], AF.Copy)
                        ah = blkp.tile([128, HID], f32, tag="ah", bufs=3)
                        nc.scalar.activation(ah[:, :], dap[:, HID:], AF.Copy)
                        dh_of_blk[b] = dh
                        ah_of_blk[b] = ah
                        sc_of_blk[b] = [ps_sc.tile([128, 128], f32, tag="sc",
                                                   name=f"sc_{l}_{b}"), 0]

                    if l == 0:
                        nc.tensor.matmul(ehp[:, j, :], ef_sb[:, j * 128:(j + 1) * 128],
                                         wsb["w4c0"][:], start=True, stop=False,
                                         skip_group_check=True)
                        nc.tensor.matmul(eemb_ps[:, j * 128:(j + 1) * 128],
                                         wsb["w_emb_e"][:],
                                         ef_sb[:, j * 128:(j + 1) * 128],
                                         start=True, stop=True,
                                         skip_group_check=True)
                    else:
                        nc.tensor.matmul(ehp[:, j, :], et_sb[:, j * 128:(j + 1) * 128],
                                         wsb[f"w4_{l}"][:], start=True,
                                         stop=False, skip_group_check=True)
                    nc.tensor.matmul(ehp[:, j, :], m23_sb[:, j * 128:(j + 1) * 128],
                                     dh_of_blk[b][:, :], start=False, stop=True,
                                     skip_group_check=True)
                if l == 0:
                    nc.scalar.activation(et_sb[:, 0:gn * 128],
                                         eemb_ps[:, 0:gn * 128], AF.Copy)
                # e_hat = (Ce + Dh[dst]) + Eh[src]: gathered Eh added on the
                # Vector engine (cheaper than an id-matmul on the PE)
                ehg = sw.tile([128, 4, HID], f32, tag="ehg")
                nc.vector.tensor_add(ehg[:, 0:gn, :], ehp[:, 0:gn, :],
                                     gtile[:, gj0:gj0 + gn, 0:HID])
                # batched sigma / mul over the group
                nc.scalar.activation(v_sb[:, 0:gn, HID:], ehg[:, 0:gn, :],
                                     AF.Sigmoid)
                nc.vector.tensor_mul(v_sb[:, 0:gn, 0:HID], v_sb[:, 0:gn, HID:],
                                     gtile[:, gj0:gj0 + gn, HID:])
                if l < L - 1:
                    nc.scalar.activation(relu_sb[:, 0:gn, :], ehg[:, 0:gn, :],
                                         AF.Relu)
                for j in range(gn):
                    tj = t + j
                    b = int(tile_blk[tj])
                    sc, nmm = sc_of_blk[b]
                    total = int(T[b, 0]) + int(T[b, 1])
                    nc.tensor.matmul(sc[:, :],
                                     m23_sb[:, gn * 128 + j * 128:gn * 128 + (j + 1) * 128],
                                     v_sb[:, j, :],
                                     start=(nmm == 0), stop=(nmm == total - 1),
                                     skip_group_check=True)
                    sc_of_blk[b][1] = nmm + 1
                    if l < L - 1:
                        nc.tensor.transpose(etr_ps[:, j * 128:(j + 1) * 128],
                                            relu_sb[:, j, :], wsb["id16"][:, :])
                    if sc_of_blk[b][1] == total:
                        update_block(b, l, sc, ah_of_blk[b])
                        del dh_of_blk[b], ah_of_blk[b], sc_of_blk[b]

                if l < L - 1:
                    nc.vector.tensor_add(enx_sb[:, 0:gn * 128],
                                         et_sb[:, 0:gn * 128],
                                         etr_ps[:, 0:gn * 128])
                    nc.sync.dma_start(out=e_buf[l % 2][:, c0:c1],
                                      in_=enx_sb[:, 0:gn * 128])

            if l < L - 1:
                allgather(l + 1)

    nc.compile()
    return nc


# ---------------------------------------------------------------------------
# entry point
# ---------------------------------------------------------------------------

_CACHE = {}


def kernel(**inputs):
    from concourse.bass_utils import run_bass_kernel_spmd

    h_feat = np.asarray(inputs["h_feat"], np.float32)
    e_feat = np.asarray(inputs["e_feat"], np.float32)
    src = np.asarray(inputs["src"])
    dst = np.asarray(inputs["dst"])
    n_nodes = h_feat.shape[0]
    n_edges = e_feat.shape[0]
    n_layers = int(np.asarray(inputs["W_layers"]).shape[0])
    cfg = _cfg(n_nodes, n_edges, n_layers)

    plan, per_core = _prep(cfg, src, dst, e_feat)
    w = _weights(cfg, inputs)

    key = ("prog", n_nodes, n_edges, n_layers, plan["ntiles"],
           tuple(plan["tile_blk"].tolist()),
           tuple(plan["chunks"]), plan["m23_cols"])
    if key not in _CACHE:
        _CACHE[key] = _build(cfg, plan)
    nc = _CACHE[key]

    npc, nblk = cfg["npc"], cfg["nblk"]
    NP = nblk * 128
    in_maps = []
    for c in range(N_CORES):
        hft = np.zeros((7, NP), np.float32)
        sl = h_feat[c * npc:(c + 1) * npc]
        hft[0:6, 0:npc] = sl.T
        hft[6, 0:npc] = 1.0
        m = per_core[c]
        im = {"h_feat_t": hft, "e_feat_t": m["e_feat_t"], "gidx": m["gidx"],
              "m23": m["m23"]}
        im.update(w)
        in_maps.append(im)

    res = run_bass_kernel_spmd(nc, in_maps, core_ids=list(range(N_CORES)))
    out = np.empty((n_nodes, 2), np.float32)
    for c in range(N_CORES):
        out[c * npc:(c + 1) * npc] = res.results[c]["out"][:, 0:npc].T
    kernel.last_results = res
    return out



# revision 22
# speedup vs baseline: 1.1591x; 1.1427x over previous
"""GatedGCN message-passing kernel for 8 TRN2 NeuronCores (Bass/Tile).

Sharding: core c owns the contiguous dst-node range [c*npc, (c+1)*npc) and all
edges whose dst falls in it.  Segment sums run per 128-node block as one-hot
matmuls on the TensorEngine (edges host-sorted by dst block); the src-side
gather uses dma_gather from a per-layer [N, 128] bf16 table of [Eh|Bh] that is
AllGathered across cores every layer.  dma_gather indices are int16, so edges
in each block are split into src<HALF / src>=HALF groups that gather from the
two halves of the table.
"""

import numpy as np

HID = 64
MLP = 128
N_CORES = 8
HALF_CAP = 32768
GCHUNK = 12
GST_BUFS = 5
KSET = 1
DBG_NO_COLLECTIVE = False
DBG_NO_GATHER = False


def _cfg(n_nodes, n_edges, n_layers):
    npc = n_nodes // N_CORES
    assert npc * N_CORES == n_nodes
    nblk = (npc + 127) // 128
    half = min(HALF_CAP, n_nodes)
    return dict(n_nodes=n_nodes, n_edges=n_edges, L=n_layers, npc=npc,
                nblk=nblk, half=half)


# ---------------------------------------------------------------------------
# host-side prep
# ---------------------------------------------------------------------------

def _prep(cfg, src, dst, e_feat):
    f16 = np.float16
    npc, nblk, half = cfg["npc"], cfg["nblk"], cfg["half"]
    n_cores = N_CORES

    src = np.asarray(src).astype(np.int64)
    dst = np.asarray(dst).astype(np.int64)
    e_feat = np.asarray(e_feat, np.float32)

    core_of = np.minimum(dst // npc, n_cores - 1)
    counts = np.zeros((n_cores, nblk, 2), np.int64)
    edge_ids = [[[None, None] for _ in range(nblk)] for _ in range(n_cores)]
    for c in range(n_cores):
        in_c = np.where(core_of == c)[0]
        dloc = dst[in_c] - c * npc
        blk = dloc // 128
        grp = (src[in_c] >= half).astype(np.int64)
        order = np.lexsort((src[in_c], grp, blk))
        in_c = in_c[order]
        key = blk[order] * 2 + grp[order]
        bounds = np.searchsorted(key, np.arange(nblk * 2 + 1))
        for b in range(nblk):
            for g in range(2):
                lo, hi = bounds[b * 2 + g], bounds[b * 2 + g + 1]
                edge_ids[c][b][g] = in_c[lo:hi]
                counts[c, b, g] = hi - lo

    T = (counts.max(axis=0) + 127) // 128          # [nblk, 2] tiles per group
    ntiles = int(T.sum())
    assert ntiles > 0

    # Slot/tile order: sets of KSET blocks, group-major inside each set, so
    # gather calls merge across blocks (one dma_gather per same-group run).
    tile_blk = np.empty(ntiles, np.int32)
    tile_grp = np.empty(ntiles, np.int32)
    tile0_bg = np.empty((nblk, 2), np.int32)       # first tile of each (b,g)
    t = 0
    for s0 in range(0, nblk, KSET):
        for g in range(2):
            for b in range(s0, min(s0 + KSET, nblk)):
                tile0_bg[b, g] = t
                for _ in range(int(T[b, g])):
                    tile_blk[t] = b
                    tile_grp[t] = g
                    t += 1

    chunks = []                                     # (tile0, ntile, group)
    s = 0
    while s < ntiles:
        g = tile_grp[s]
        e = s
        while e < ntiles and tile_grp[e] == g and e - s < GCHUNK:
            e += 1
        chunks.append((int(s), int(e - s), int(g)))
        s = e

    # Processing groups in block-major order (all of b's tiles consecutively)
    # so each block's PSUM accumulation opens and closes without another
    # block's accumulation interleaving in the same bank.  Groups never
    # straddle a (b,g) run nor a gather-chunk boundary (one gst buffer per
    # group).
    chunk_end = np.empty(ntiles, np.int32)
    for (c0, cn, _g) in chunks:
        chunk_end[c0:c0 + cn] = c0 + cn
    groups = []   # (t0, gn, m23_off_cols)
    off = 0
    for b in range(nblk):
        for g in range(2):
            t = int(tile0_bg[b, g])
            end = t + int(T[b, g])
            while t < end:
                gn = min(4, end - t, int(chunk_end[t]) - t)
                groups.append((int(t), int(gn), int(off)))
                off += 2 * gn * 128
                t += gn
    plan = dict(T=T, ntiles=ntiles, tile_blk=tile_blk, tile_grp=tile_grp,
                chunks=chunks, groups=groups, m23_cols=int(off))

    E_pad = ntiles * 128
    rows = np.arange(E_pad) % 128
    tt = np.arange(E_pad) // 128
    per_core = []
    for c in range(n_cores):
        eid = np.zeros(E_pad, np.int64)
        valid = np.zeros(E_pad, bool)
        pos = 0
        for s0 in range(0, nblk, KSET):
            for g in range(2):
                for b in range(s0, min(s0 + KSET, nblk)):
                    ids = edge_ids[c][b][g]
                    n = len(ids)
                    eid[pos:pos + n] = ids
                    valid[pos:pos + n] = True
                    pos += int(T[b, g]) * 128
        esrc = src[eid].copy()
        edst = (dst[eid] - c * npc).copy()
        esrc[~valid] = 0
        edst[~valid] = 0

        bf16 = f16
        gi = esrc.copy()
        gi[gi >= half] -= half
        gi = gi.astype(np.int16).reshape(-1, 16).T
        gidx = np.ascontiguousarray(np.tile(gi, (8, 1)))        # [128, E_pad/16]

        doff = edst - tile_blk[tt].astype(np.int64) * 128
        ok = valid & (doff >= 0) & (doff < 128)
        m3 = np.zeros((ntiles, 128, 128), bf16)                  # [e, n]
        m3[tt[ok], rows[ok], doff[ok]] = 1.0
        m2f = m3.transpose(2, 0, 1).reshape(128, E_pad)          # [n, tiles*e]
        m3f = m3.transpose(1, 0, 2).reshape(128, E_pad)          # [e, tiles*n]
        # group-interleaved per the plan's group list: [m2 cols | m3 cols]
        import ml_dtypes
        m23 = np.zeros((128, plan["m23_cols"]), ml_dtypes.float8_e4m3)
        for (gt0, gg, goff) in plan["groups"]:
            a, b_ = gt0 * 128, (gt0 + gg) * 128
            m23[:, goff:goff + (b_ - a)] = m2f[:, a:b_]
            m23[:, goff + gg * 128:goff + 2 * gg * 128] = m3f[:, a:b_]

        ef = np.zeros((3, E_pad), f16)
        efv = e_feat[eid]
        efv[~valid] = 0.0
        ef[0, :] = efv[:, 0].astype(f16)
        ef[1, :] = efv[:, 1].astype(f16)
        ef[2, :] = valid.astype(f16)

        per_core.append(dict(gidx=gidx, m23=m23, e_feat_t=ef))
    return plan, per_core


def _weights(cfg, inputs):
    f16 = np.float16
    f32 = np.float32
    Lw = np.asarray(inputs["W_layers"], f32)
    Lb = np.asarray(inputs["b_layers"], f32)
    w_emb_e = np.asarray(inputs["W_emb_e"], f32)
    b_emb_e = np.asarray(inputs["b_emb_e"], f32)
    w = {}
    w["w_emb_h"] = np.concatenate(
        [np.asarray(inputs["W_emb_h"], f32),
         np.asarray(inputs["b_emb_h"], f32)[None, :]], 0)           # [7,64] f32
    w["w_emb_e"] = np.concatenate(
        [w_emb_e, b_emb_e[None, :]], 0).astype(f16)
    for l in range(cfg["L"]):
        A, B, D, E, C = (Lw[l, i] for i in range(5))
        bA, bB, bD, bE, bC = (Lb[l, i] for i in range(5))
        eb = np.zeros((HID + 1, 2 * HID), f32)
        eb[:HID, :HID] = E
        eb[:HID, HID:] = B
        eb[HID, HID:] = bB
        w[f"w_eb{l}"] = eb
        da = np.zeros((HID + 1, 2 * HID), f32)
        da[:HID, :HID] = D
        da[:HID, HID:] = A
        da[HID, :HID] = bD + bC + bE
        da[HID, HID:] = bA
        w[f"w_da{l}"] = da
        if l == 0:
            w["w4c0"] = np.concatenate(
                [w_emb_e @ C, (b_emb_e @ C)[None, :]], 0).astype(f16)
        else:
            w[f"w4_{l}"] = C.astype(f16)                           # [64,64]
    w["w1"] = np.concatenate(
        [np.asarray(inputs["W1"], f32),
         np.asarray(inputs["b1"], f32)[None, :]], 0)                # [65,128]
    w["w2"] = np.asarray(inputs["W2"], f32).astype(f16)            # [128,2]
    w["b2"] = np.asarray(inputs["b2"], f32).reshape(2, 1)           # [2,1]
    ident = np.eye(128)
    w["id16"] = ident.astype(f16)
    w["id32"] = ident.astype(f32)
    return w


# ---------------------------------------------------------------------------
# device program
# ---------------------------------------------------------------------------

def _build(cfg, plan):
    import concourse.bacc as bacc
    import concourse.mybir as mybir
    from concourse import tile
    from contextlib import ExitStack

    f32 = mybir.dt.float32
    f16 = mybir.dt.float16
    i16 = mybir.dt.int16
    AF = mybir.ActivationFunctionType
    ALU = mybir.AluOpType

    L = cfg["L"]
    npc, nblk, half = cfg["npc"], cfg["nblk"], cfg["half"]
    ntiles = plan["ntiles"]
    tile_blk = plan["tile_blk"]
    chunks = plan["chunks"]
    T = plan["T"]
    E_pad = ntiles * 128
    NP = nblk * 128
    n_tab = npc * N_CORES

    nc = bacc.Bacc("TRN2", target_bir_lowering=False, debug=False,
                   num_devices=N_CORES, num_swdge_queues=4)

    wshapes = {
        "w_emb_h": ([7, HID], f32), "w_emb_e": ([3, HID], f16),
        "w4c0": ([3, HID], f16), "w1": ([HID + 1, MLP], f32),
        "w2": ([MLP, 2], f16), "b2": ([2, 1], f32),
        "id16": ([128, 128], f16), "id32": ([128, 128], f32),
    }
    for l in range(L):
        wshapes[f"w_eb{l}"] = ([HID + 1, 2 * HID], f32)
        wshapes[f"w_da{l}"] = ([HID + 1, 2 * HID], f32)
        if l > 0:
            wshapes[f"w4_{l}"] = ([HID, HID], f16)

    p_hfeat = nc.declare_dram_parameter("h_feat_t", [7, NP], f32, isOutput=False)
    p_efeat = nc.declare_dram_parameter("e_feat_t", [3, E_pad], f16, isOutput=False)
    p_gidx = nc.declare_dram_parameter("gidx", [128, E_pad // 16], i16, isOutput=False)
    p_m23 = nc.declare_dram_parameter("m23", [128, plan["m23_cols"]], mybir.dt.float8e4, isOutput=False)
    p_w = {k: nc.declare_dram_parameter(k, s, d, isOutput=False)
           for k, (s, d) in wshapes.items()}
    p_out = nc.declare_dram_parameter("out", [2, NP], f32, isOutput=True)

    eb_own = [nc.dram_tensor(f"eb_own{i}", [npc, 2 * HID], f16) for i in range(2)]
    eb_tab = [nc.dram_tensor(f"eb_tab{i}", [n_tab, 2 * HID], f16,
                             addr_space="Shared") for i in range(2)]
    e_buf = [nc.dram_tensor(f"e_buf{i}", [HID, E_pad], f16)
             for i in range(2)]
    rg = [list(range(N_CORES))]

    with tile.TileContext(nc) as tc, ExitStack() as ctx:
        const = ctx.enter_context(tc.tile_pool(name="const", bufs=1))
        persist = ctx.enter_context(tc.tile_pool(name="persist", bufs=1))
        sw = ctx.enter_context(tc.tile_pool(name="sw", bufs=4))
        gst = ctx.enter_context(tc.tile_pool(name="gst", bufs=GST_BUFS))
        blkp = ctx.enter_context(tc.tile_pool(name="blkp", bufs=2))
        ps_eh = ctx.enter_context(tc.tile_pool(name="ps_eh", bufs=2, space="PSUM"))
        ps_sc = ctx.enter_context(tc.tile_pool(name="ps_sc", bufs=2, space="PSUM"))
        ps_tr = ctx.enter_context(tc.tile_pool(name="ps_tr", bufs=2, space="PSUM"))
        ps_bk = ctx.enter_context(tc.tile_pool(name="ps_bk", bufs=2, space="PSUM"))

        wsb = {}
        for k, (s, d) in wshapes.items():
            t_ = const.tile(s, d, tag=f"w_{k}")
            nc.sync.dma_start(out=t_[:], in_=p_w[k][:, :])
            wsb[k] = t_

        gidx_sb = persist.tile([128, E_pad // 16], i16)
        nc.sync.dma_start(out=gidx_sb[:, :], in_=p_gidx[:, :])

        h_sb = persist.tile([128, nblk * HID], f32)
        ht_sb = persist.tile([HID + 1, NP], f32)
        nc.vector.memset(ht_sb[HID:HID + 1, :], 1.0)
        hfeat_sb = persist.tile([7, NP], f32)
        nc.sync.dma_start(out=hfeat_sb[:, :], in_=p_hfeat[:, :])

        def ht_block(b):
            return ht_sb[:, b * 128:(b + 1) * 128]

        def transpose_h_and_table(b, l):
            trp = ps_tr.tile([HID, 512], f32, tag="tr")
            nc.tensor.transpose(trp[:, 0:128], h_sb[:, b * HID:(b + 1) * HID],
                                wsb["id32"][:, :])
            nc.scalar.activation(ht_sb[0:HID, b * 128:(b + 1) * 128],
                                 trp[:, 0:128], AF.Copy)
            if l < L:
                ebp = ps_bk.tile([128, 128], f32, tag="bk")
                nc.tensor.matmul(ebp[:, :], ht_block(b), wsb[f"w_eb{l}"][:],
                                 start=True, stop=True, skip_group_check=True)
                ebs = blkp.tile([128, 2 * HID], f16, tag="ebs")
                nc.scalar.activation(ebs[:, :], ebp[:, :], AF.Copy)
                ne = min(128, npc - b * 128)
                nc.sync.dma_start(out=eb_own[l % 2][b * 128:b * 128 + ne, :],
                                  in_=ebs[0:ne, :])

        def head_block(b):
            z1p = ps_bk.tile([128, 128], f32, tag="bk")
            nc.tensor.matmul(z1p[:, :], wsb["w1"][:], ht_block(b),
                             start=True, stop=True, skip_group_check=True)
            z1 = blkp.tile([MLP, 128], f16, tag="z1s")
            nc.scalar.activation(z1[:, :], z1p[:, :], AF.Relu)
            z2p = ps_tr.tile([HID, 512], f32, tag="tr")
            nc.tensor.matmul(z2p[0:2, 0:128], wsb["w2"][:], z1[:, :],
                             start=True, stop=True, skip_group_check=True)
            th = blkp.tile([2, 128], f32, tag="th")
            nc.scalar.activation(th[:, :], z2p[0:2, 0:128], AF.Tanh,
                                 bias=wsb["b2"][:, 0:1])
            out_sb = blkp.tile([2, 128], f32, tag="outs")
            nc.vector.tensor_scalar_mul(out_sb[:, :], th[:, :], -1.2)
            nc.sync.dma_start(out=p_out[:, b * 128:(b + 1) * 128], in_=out_sb[:, :])

        def update_block(b, l, sc, ah):
            hb = h_sb[:, b * HID:(b + 1) * HID]
            den = blkp.tile([128, HID], f32, tag="den")
            nc.scalar.activation(den[:, :], sc[:, HID:], AF.Copy, bias=1e-6)
            rec = blkp.tile([128, HID], f32, tag="rec")
            nc.vector.reciprocal(rec[:, :], den[:, :])
            div = blkp.tile([128, HID], f32, tag="div")
            nc.vector.tensor_mul(div[:, :], sc[:, 0:HID], rec[:, :])
            pre = blkp.tile([128, HID], f32, tag="pre")
            nc.vector.tensor_add(pre[:, :], div[:, :], ah[:, :])
            rl = blkp.tile([128, HID], f32, tag="rl")
            nc.scalar.activation(rl[:, :], pre[:, :], AF.Relu)
            nc.vector.tensor_add(hb, hb, rl[:, :])
            transpose_h_and_table(b, l + 1)
            if l + 1 == L:
                head_block(b)

        # ---- layer 0: h embedding + transposed copy + EB table -----------
        for b in range(nblk):
            ps = ps_bk.tile([128, 128], f32, tag="bk")
            nc.tensor.matmul(ps[:, 0:HID], hfeat_sb[:, b * 128:(b + 1) * 128],
                             wsb["w_emb_h"][:], start=True, stop=True,
                             skip_group_check=True)
            nc.scalar.activation(h_sb[:, b * HID:(b + 1) * HID], ps[:, 0:HID],
                                 AF.Copy)
            transpose_h_and_table(b, 0)

        def allgather(l=0):
            if DBG_NO_COLLECTIVE:
                cp = blkp.tile([128, 2 * HID], f16, tag="agcp", name=f"agcp{len(ag_n)}")
                ag_n.append(1)
                nc.sync.dma_start(out=cp[:, :], in_=eb_own[l % 2][0:128, :])
                nc.sync.dma_start(out=eb_tab[l % 2][0:128, :], in_=cp[:, :])
                return
            nc.gpsimd.collective_compute(
                "AllGather", ALU.bypass, replica_groups=rg,
                ins=[eb_own[l % 2][:, :].opt()], outs=[eb_tab[l % 2][:, :].opt()])
        ag_n = []

        allgather(0)

        # ---- layer sweeps -------------------------------------------------
        for l in range(L):
            g_of_tile = {}
            for ci, (t0, tn, grp) in enumerate(chunks):
                g = gst.tile([128, GCHUNK, 128], f16, tag="gather")
                base = half if grp else 0
                nrows = (n_tab - half) if grp else half
                if DBG_NO_GATHER:
                    nc.vector.memset(g[:, 0:tn, :], 0.125)
                else:
                    nc.gpsimd.dma_gather(
                        out_ap=g[:, 0:tn, :],
                        in_ap=eb_tab[l % 2][base:base + nrows, :],
                        idxs_ap=gidx_sb[:, t0 * 8:(t0 + tn) * 8],
                        num_idxs=tn * 128,
                        num_idxs_reg=tn * 128,
                        elem_size=2 * HID,
                        single_packet=False,
                        queue_num=ci % 4,
                    )
                for j in range(tn):
                    g_of_tile[t0 + j] = (g, j)

            dh_of_blk, ah_of_blk, sc_of_blk = {}, {}, {}
            sc_set_of = {}
            for (t, gn, goff) in plan["groups"]:
                c0, c1 = t * 128, (t + gn) * 128
                gtile, gj0 = g_of_tile[t]
                ehp = ps_eh.tile([128, 4, HID], f32, tag="ehat")
                et_sb = sw.tile([HID, 512], f16, tag="et")
                if l == 0:
                    ef_sb = sw.tile([3, 512], f16, tag="ef")
                    nc.sync.dma_start(out=ef_sb[:, 0:gn * 128],
                                      in_=p_efeat[:, c0:c1])
                    eemb_ps = ps_tr.tile([HID, 512], f32, tag="tr")
                else:
                    nc.sync.dma_start(out=et_sb[:, 0:gn * 128],
                                      in_=e_buf[(l - 1) % 2][:, c0:c1])
                m23_sb = sw.tile([128, 1024], mybir.dt.float8e4, tag="m23")
                nc.sync.dma_start(out=m23_sb[:, 0:2 * gn * 128],
                                  in_=p_m23[:, goff:goff + 2 * gn * 128])
                v_sb = sw.tile([128, 4, 128], f16, tag="v")
                if l < L - 1:
                    etr_ps = ps_tr.tile([HID, 512], f16, tag="tr")
                    relu_sb = sw.tile([128, 4, HID], f16, tag="relu")
                    enx_sb = sw.tile([HID, 512], f16, tag="enx")

                for j in range(gn):
                    tj = t + j
                    b = int(tile_blk[tj])
                    if b not in dh_of_blk:
                        dap = ps_bk.tile([128, 128], f32, tag="bk")
                        nc.tensor.matmul(dap[:, :], ht_block(b),
                                         wsb[f"w_da{l}"][:], start=True,
                                         stop=True, skip_group_check=True)
                        dh = blkp.tile([128, HID], f16, tag="dh", bufs=3)
                        nc.scalar.activation(dh[:, :], dap[:, 0:HID], AF.Copy)
                        ah = blkp.tile([128, HID], f32, tag="ah", bufs=3)
                        nc.scalar.activation(ah[:, :], dap[:, HID:], AF.Copy)
                        dh_of_blk[b] = dh
                        ah_of_blk[b] = ah
                        sc_of_blk[b] = [ps_sc.tile([128, 128], f32, tag="sc",
                                                   name=f"sc_{l}_{b}"), 0]

                    if l == 0:
                        nc.tensor.matmul(ehp[:, j, :], ef_sb[:, j * 128:(j + 1) * 128],
                                         wsb["w4c0"][:], start=True, stop=False,
                                         skip_group_check=True)
                        nc.tensor.matmul(eemb_ps[:, j * 128:(j + 1) * 128],
                                         wsb["w_emb_e"][:],
                                         ef_sb[:, j * 128:(j + 1) * 128],
                                         start=True, stop=True,
                                         skip_group_check=True)
                    else:
                        nc.tensor.matmul(ehp[:, j, :], et_sb[:, j * 128:(j + 1) * 128],
                                         wsb[f"w4_{l}"][:], start=True,
                                         stop=False, skip_group_check=True)
                    nc.tensor.matmul(ehp[:, j, :], m23_sb[:, j * 128:(j + 1) * 128],
                                     dh_of_blk[b][:, :], start=False, stop=True,
                                     skip_group_check=True)
                if l == 0:
                    nc.scalar.activation(et_sb[:, 0:gn * 128],
                                         eemb_ps[:, 0:gn * 128], AF.Copy)
                # e_hat = (Ce + Dh[dst]) + Eh[src]: gathered Eh added on the
                # Vector engine (cheaper than an id-matmul on the PE)
                ehg = sw.tile([128, 4, HID], f32, tag="ehg")
                nc.vector.tensor_add(ehg[:, 0:gn, :], ehp[:, 0:gn, :],
                                     gtile[:, gj0:gj0 + gn, 0:HID])
                # batched sigma / mul over the group
                nc.scalar.activation(v_sb[:, 0:gn, HID:], ehg[:, 0:gn, :],
                                     AF.Sigmoid)
                nc.vector.tensor_mul(v_sb[:, 0:gn, 0:HID], v_sb[:, 0:gn, HID:],
                                     gtile[:, gj0:gj0 + gn, HID:])
                if l < L - 1:
                    nc.scalar.activation(relu_sb[:, 0:gn, :], ehg[:, 0:gn, :],
                                         AF.Relu)
                for j in range(gn):
                    tj = t + j
                    b = int(tile_blk[tj])
                    sc, nmm = sc_of_blk[b]
                    total = int(T[b, 0]) + int(T[b, 1])
                    nc.tensor.matmul(sc[:, :],
                                     m23_sb[:, gn * 128 + j * 128:gn * 128 + (j + 1) * 128],
                                     v_sb[:, j, :],
                                     start=(nmm == 0), stop=(nmm == total - 1),
                                     skip_group_check=True)
                    sc_of_blk[b][1] = nmm + 1
                    if l < L - 1:
                        nc.tensor.transpose(etr_ps[:, j * 128:(j + 1) * 128],
                                            relu_sb[:, j, :], wsb["id16"][:, :])
                    if sc_of_blk[b][1] == total:
                        update_block(b, l, sc, ah_of_blk[b])
                        del dh_of_blk[b], ah_of_blk[b], sc_of_blk[b]

                if l < L - 1:
                    nc.vector.tensor_add(enx_sb[:, 0:gn * 128],
                                         et_sb[:, 0:gn * 128],
                                         etr_ps[:, 0:gn * 128])
                    nc.sync.dma_start(out=e_buf[l % 2][:, c0:c1],
                                      in_=enx_sb[:, 0:gn * 128])

            if l < L - 1:
                allgather(l + 1)

    nc.compile()
    return nc


# ---------------------------------------------------------------------------
# entry point
# ---------------------------------------------------------------------------

_CACHE = {}


def kernel(**inputs):
    from concourse.bass_utils import run_bass_kernel_spmd

    h_feat = np.asarray(inputs["h_feat"], np.float32)
    e_feat = np.asarray(inputs["e_feat"], np.float32)
    src = np.asarray(inputs["src"])
    dst = np.asarray(inputs["dst"])
    n_nodes = h_feat.shape[0]
    n_edges = e_feat.shape[0]
    n_layers = int(np.asarray(inputs["W_layers"]).shape[0])
    cfg = _cfg(n_nodes, n_edges, n_layers)

    plan, per_core = _prep(cfg, src, dst, e_feat)
    w = _weights(cfg, inputs)

    key = ("prog", n_nodes, n_edges, n_layers, plan["ntiles"],
           tuple(plan["tile_blk"].tolist()),
           tuple(plan["chunks"]), plan["m23_cols"])
    if key not in _CACHE:
        _CACHE[key] = _build(cfg, plan)
    nc = _CACHE[key]

    npc, nblk = cfg["npc"], cfg["nblk"]
    NP = nblk * 128
    in_maps = []
    for c in range(N_CORES):
        hft = np.zeros((7, NP), np.float32)
        sl = h_feat[c * npc:(c + 1) * npc]
        hft[0:6, 0:npc] = sl.T
        hft[6, 0:npc] = 1.0
        m = per_core[c]
        im = {"h_feat_t": hft, "e_feat_t": m["e_feat_t"], "gidx": m["gidx"],
              "m23": m["m23"]}
        im.update(w)
        in_maps.append(im)

    res = run_bass_kernel_spmd(nc, in_maps, core_ids=list(range(N_CORES)))
    out = np.empty((n_nodes, 2), np.float32)
    for c in range(N_CORES):
        out[c * npc:(c + 1) * npc] = res.results[c]["out"][:, 0:npc].T
    kernel.last_results = res
    return out

